# revision 10
# baseline (speedup 1.0000x reference)
"""Trainium2 Bass kernel for nn_ANN_Comp_29240137351521 (dense_cnn).

Reference computes, per batch row b of x [16384, 512] (complex, given as
real/imag f32 pairs):
    h = x @ w0                      # [B, 512] complex
    a = ifft(fft(h, n=1023)^2)      # full self-convolution, [B, 1023]
    out = |a @ wlast|               # [B, 10] f32

Algebraic collapse used here: the self-convolution + final contraction is a
polynomial-evaluation identity. With L = 1024 >= 2*512-1 evaluation points at
the L-th roots of unity:
    e   = x @ F        where F  = fft(w0, n=L, axis=1)        [512, L]
    z   = (e*e) @ Wt   where Wt = ifft(pad(wlast, L), axis=0) [L, 10]
    out = |z|
so the whole network is two dense matmuls + an elementwise complex square --
no FFT on device. F and Wt are tiny weight transforms folded on the host.

Real-expanded form computed on device (per core, data-parallel over batch):
    er = xr@Fr - xi@Fi ;  ei = xr@Fi + xi@Fr          (PSUM accumulation)
    sr = er^2 - ei^2   ;  w  = er*ei                  (ACT squares + DVE)
    [zr | zi] = sr @ [Wtr|Wti] + w @ [-2Wti|2Wtr]     (stacked second matmul,
                                                       4-way column-packed)
    host: out = sqrt(zr^2 + zi^2)

Everything runs transposed (l on partitions, batch on the free axis) so the
second matmul needs no on-device transpose; x is fed pre-transposed from the
host in bf16 (measured end-to-end error 4e-3 of output scale, ~5x under the
2e-2 gate; squares/accumulations stay fp32). Weights and activations are
host-packed into [128, *] layouts so each input is one fat contiguous DMA on
a hardware DGE queue; dummy warm-up matmuls run during the load phase so the
PE HAM clock-gate is released before real work arrives.

Sharding: pure data parallel -- batch split 8 ways, weights replicated.
"""

import numpy as np
import ml_dtypes

import concourse.bass as bass
import concourse.mybir as mybir
from concourse import bacc, tile
from concourse.bass_utils import run_bass_kernel_spmd

NCORES = 8
B, D, L, C = 16384, 512, 1024, 10
BC = B // NCORES          # batch per core = 2048
P = 128                   # partitions
BN = 512                  # batch columns per PSUM tile
ND = D // P               # 4 contraction chunks
NL = L // P               # 8 output-l chunks
NB = BC // BN             # 4 batch positions per core

F32 = mybir.dt.float32
BF16 = mybir.dt.bfloat16

_NC_CACHE = None


def build_nc():
    """Build (once) the single-core Bass graph; SPMD-replicated to 8 cores."""
    global _NC_CACHE
    if _NC_CACHE is not None:
        return _NC_CACHE

    nc = bacc.Bacc(None, target_bir_lowering=False)

    # All inputs host-packed to [128, *]: column p is SBUF partition p.
    xtr_d = nc.declare_dram_parameter("xT_r", [P, ND * BC], BF16, isOutput=False)
    xti_d = nc.declare_dram_parameter("xT_i", [P, ND * BC], BF16, isOutput=False)
    fr_d = nc.declare_dram_parameter("F_r", [P, ND * L], BF16, isOutput=False)
    fi_d = nc.declare_dram_parameter("F_i", [P, ND * L], BF16, isOutput=False)
    fin_d = nc.declare_dram_parameter("F_in", [P, ND * L], BF16, isOutput=False)
    wa_d = nc.declare_dram_parameter("WtA", [P, NL * 2 * C], BF16, isOutput=False)
    wb_d = nc.declare_dram_parameter("WtB", [P, NL * 2 * C], BF16, isOutput=False)
    out_d = nc.declare_dram_parameter("out", [8 * C, BC], F32, isOutput=True)

    with tile.TileContext(nc) as tc:
        with (
            tc.tile_pool(name="wts", bufs=1) as wts,
            tc.tile_pool(name="xs", bufs=1) as xs,
            tc.tile_pool(name="tmp", bufs=3) as tmp,
            tc.tile_pool(name="sq", bufs=3) as sq,
            tc.tile_pool(name="zo", bufs=2) as zo,
            tc.tile_pool(name="pse", bufs=3, space="PSUM") as pse,
            tc.tile_pool(name="psz", bufs=2, space="PSUM") as psz,
        ):
            # --- PE warm-up: release the HAM clock gate during load ---
            dummy = wts.tile([P, 64], BF16, tag="dummy")
            nc.gpsimd.memset(dummy[:], 0.0)
            wacc = pse.tile([64, 64], F32, tag="er")
            for i in range(64):
                nc.tensor.matmul(wacc[:], dummy[:, 0:64], dummy[:],
                                 start=(i == 0), stop=(i == 63),
                                 skip_group_check=True)

            # --- inputs: fine-grained prioritized DMA on the two HWDGE
            # queues; chunk (b0, l0) data lands first -------------------------
            fr = wts.tile([P, ND * L], BF16, tag="fr")
            fi = wts.tile([P, ND * L], BF16, tag="fi")
            fin = wts.tile([P, ND * L], BF16, tag="fin")
            xtr = xs.tile([P, ND * BC], BF16, tag="xtr")
            xti = xs.tile([P, ND * BC], BF16, tag="xti")
            wa = wts.tile([P, NL * 2 * C], BF16, tag="wa")
            wb = wts.tile([P, NL * 2 * C], BF16, tag="wb")

            def fchunk(d):      # F columns for contraction chunk d
                return slice(d * L, (d + 1) * L)

            def xsl(d, b):      # x chunk [128, 512]: (d, batch-pos b)
                return slice(d * BC + b * BN, d * BC + (b + 1) * BN)

            def xrest(d):       # x columns b1..b3 of chunk d
                return slice(d * BC + BN, (d + 1) * BC)

            for d in range(ND):
                nc.sync.dma_start(fr[:, fchunk(d)], fr_d[:, fchunk(d)])
                nc.sync.dma_start(xtr[:, xsl(d, 0)], xtr_d[:, xsl(d, 0)])
                nc.scalar.dma_start(xti[:, xsl(d, 0)], xti_d[:, xsl(d, 0)])
                nc.scalar.dma_start(fi[:, fchunk(d)], fi_d[:, fchunk(d)])
            for d in range(ND):
                nc.scalar.dma_start(fin[:, fchunk(d)], fin_d[:, fchunk(d)])
            nc.sync.dma_start(wa[:], wa_d[:])
            nc.sync.dma_start(wb[:], wb_d[:])
            for d in range(ND):
                nc.sync.dma_start(xtr[:, xrest(d)], xtr_d[:, xrest(d)])
                nc.scalar.dma_start(xti[:, xrest(d)], xti_d[:, xrest(d)])

            def fsl(d, l):      # F chunk [128, 128]: weight (d, l)
                return slice(d * L + l * P, d * L + (l + 1) * P)

            def wsl(l):         # Wt chunk [128, 20]
                return slice(l * 2 * C, (l + 1) * 2 * C)

            # --- main pipeline ----------------------------------------------
            # z-matmuls are emitted one chunk late (pending list) so the PE
            # never waits on the DVE producing sr/w; each flush is 4
            # column-packed matmuls that run concurrently in the array.
            pending = []        # (zz, j, wt, wtslice, rhs, start, stop)
            for b in range(NB):
                bs = slice(b * BN, (b + 1) * BN)
                zz = psz.tile([P, BN], F32, tag="zz")
                for l in range(NL):
                    er = pse.tile([P, BN], F32, tag="er")
                    ei = pse.tile([P, BN], F32, tag="ei")
                    # fr[d] serves two matmuls back-to-back (weight reuse)
                    for d in range(ND):
                        nc.tensor.matmul(
                            er[:], fr[:, fsl(d, l)], xtr[:, xsl(d, b)],
                            start=(d == 0), stop=False, skip_group_check=True)
                        nc.tensor.matmul(
                            ei[:], fr[:, fsl(d, l)], xti[:, xsl(d, b)],
                            start=(d == 0), stop=False, skip_group_check=True)
                    for d in range(ND):
                        nc.tensor.matmul(
                            ei[:], fi[:, fsl(d, l)], xtr[:, xsl(d, b)],
                            start=False, stop=(d == ND - 1),
                            skip_group_check=True)
                    for d in range(ND):
                        nc.tensor.matmul(
                            er[:], fin[:, fsl(d, l)], xti[:, xsl(d, b)],
                            start=False, stop=(d == ND - 1),
                            skip_group_check=True)

                    if len(pending) >= 6:
                        for (pzz, j, wt, ws, rhs) in pending[:4]:
                            nc.tensor.matmul(
                                pzz[32 * j:32 * j + 2 * C, :], wt[:, ws],
                                rhs[:],
                                start=(ws.start < 4 * C),
                                stop=(ws.start >= (NL - 2) * 2 * C),
                                tile_position=(0, 32 * j),
                                skip_group_check=True)
                        pending = pending[4:]

                    # squares: u = er^2, v = ei^2, ci = ei on ACT;
                    # sr = u - v, w = er * ci on DVE (bf16 outputs)
                    u = tmp.tile([P, BN], F32, tag="u")
                    nc.scalar.square(u[:], er[:])
                    v = tmp.tile([P, BN], F32, tag="v")
                    nc.scalar.square(v[:], ei[:])
                    ci = tmp.tile([P, BN], F32, tag="ci")
                    nc.scalar.copy(ci[:], ei[:])
                    sr = sq.tile([P, BN], BF16, tag="sr")
                    nc.vector.tensor_sub(sr[:], u[:], v[:])
                    w = sq.tile([P, BN], BF16, tag="w")
                    nc.vector.tensor_mul(w[:], er[:], ci[:])

                    for t, (wt, rhs) in enumerate(((wa, sr), (wb, w))):
                        j = (2 * l + t) % 4
                        pending.append((zz, j, wt, wsl(l), rhs))

                # flush remaining z-matmuls for this b, then stream out the
                # four col-group slices (host sums them)
                for (pzz, j, wt, ws, rhs) in pending:
                    nc.tensor.matmul(
                        pzz[32 * j:32 * j + 2 * C, :], wt[:, ws], rhs[:],
                        start=(ws.start < 4 * C),
                        stop=(ws.start >= (NL - 2) * 2 * C),
                        tile_position=(0, 32 * j),
                        skip_group_check=True)
                pending = []
                for j in range(4):
                    zt = zo.tile([2 * C, BN], F32, tag=f"zt{j}")
                    nc.scalar.copy(zt[:], zz[32 * j:32 * j + 2 * C, :])
                    nc.sync.dma_start(out_d[2 * C * j:2 * C * (j + 1), bs],
                                      zt[:])

    nc.compile()
    _NC_CACHE = nc
    return nc


def _pack128(a):
    """[R*128, M] -> [128, R*M] so column p is SBUF partition p."""
    R = a.shape[0] // P
    return np.ascontiguousarray(
        a.reshape(R, P, a.shape[1]).transpose(1, 0, 2).reshape(P, -1))


def _host_weights(w0_real, w0_imag, wlast_real, wlast_imag):
    w0 = w0_real.astype(np.float64) + 1j * w0_imag.astype(np.float64)
    wl = wlast_real.astype(np.float64) + 1j * wlast_imag.astype(np.float64)
    F = np.fft.fft(w0, n=L, axis=1)                       # [512, 1024]
    Wt = np.fft.ifft(
        np.concatenate([wl, np.zeros((1, C))], axis=0), axis=0)  # [1024, 10]
    bf = ml_dtypes.bfloat16
    Fr = _pack128(F.real.astype(bf))
    Fi = _pack128(F.imag.astype(bf))
    Fin = _pack128((-F.imag).astype(bf))
    Wtr, Wti = Wt.real, Wt.imag
    WtA = _pack128(np.hstack([Wtr, Wti]).astype(bf))
    WtB = _pack128(np.hstack([-2 * Wti, 2 * Wtr]).astype(bf))
    return Fr, Fi, Fin, WtA, WtB


def make_in_maps(x_real, x_imag, w0_real, w0_imag, wlast_real, wlast_imag):
    Fr, Fi, Fin, WtA, WtB = _host_weights(
        w0_real, w0_imag, wlast_real, wlast_imag)
    bf = ml_dtypes.bfloat16
    xr = np.ascontiguousarray(x_real.T, dtype=bf)   # [512, 16384]
    xi = np.ascontiguousarray(x_imag.T, dtype=bf)
    in_maps = []
    for c in range(NCORES):
        sl = slice(c * BC, (c + 1) * BC)
        in_maps.append({
            "xT_r": _pack128(xr[:, sl]),
            "xT_i": _pack128(xi[:, sl]),
            "F_r": Fr, "F_i": Fi, "F_in": Fin, "WtA": WtA, "WtB": WtB,
        })
    return in_maps


def postprocess(results):
    """results: list of per-core dicts with 'out' [20, BC] -> [B, C] f32."""
    outs = []
    for c in range(NCORES):
        o = results[c]["out"]                             # [8C, BC]
        z = o.reshape(4, 2 * C, BC).sum(axis=0)           # [2C, BC]
        mag = np.sqrt(z[:C] ** 2 + z[C:2 * C] ** 2).T     # [BC, 10]
        outs.append(mag)
    return np.ascontiguousarray(np.concatenate(outs, axis=0), dtype=np.float32)


def kernel(x_real, x_imag, w0_real, w0_imag, wlast_real, wlast_imag):
    nc = build_nc()
    in_maps = make_in_maps(
        x_real, x_imag, w0_real, w0_imag, wlast_real, wlast_imag)
    res = run_bass_kernel_spmd(nc, in_maps, core_ids=list(range(NCORES)))
    return postprocess(res.results)
